# revision 26
# baseline (speedup 1.0000x reference)
"""Trainium2 Bass kernel for multi-head attention (B=2, S=2048, D=1024, H=16).

Sharding: 8 cores = 2 batches x 4 head-groups (4 heads = 256 dims per core).
Tensor-parallel split of W_q/W_k/W_v column-wise, W_o row-wise; partial
outputs summed on host (free), plus data-parallel over batch.

Structure (final):
  - b_k dropped (softmax shift-invariance: q.b_k is constant over keys);
    b_v folded into b_o on host (attention weights sum to 1, so
    ctx = ctx' + b_v and b_v @ W_o.T is a constant vector).
  - Scores^T [k, q] per key-tile as a row-group-packed concurrent pair
    (K=64 per head); exp on ScalarE over [128,1024] PSUM tiles
    (~1.13us each) paces the whole stream.
  - AV and the softmax denominator are column-tiled concurrent matmul
    pairs accumulating into one [128,2,512] PSUM tile; the ones-matmul
    leaves den duplicated across each head's 64 partitions, aligned
    with ctx.
  - Drain chain per block is just two DVE ops: reciprocal_approx_fast
    of the duplicated den rows, then one tensor_tensor multiply that
    normalizes ctx during its PSUM->SBUF drain.  The output projection
    then contracts K=128 (both heads) and accumulates both head-pairs
    into a single PSUM tile per (q-tile, e-half) -> single drain.
  - Global-slot software pipeline: slot s = scores+exp of tile s; AVs
    follow a lag ladder (+6:[0,1], +7:[2,3], +8..+19:[4..15]) so each
    block's drain chain is fully emitted before the shared cd banks are
    re-allocated; the last block compresses its tail AVs.  Projections
    (DMA-ordered K, Q(qc0), V, Q(rest)) and the previous qc's output
    projection are statically interleaved into slots with spare PE time.
  - bq rides the SWDGE queue (its [128,2] layout is 128 tiny
    descriptors); first x chunks ride the scalar HWDGE ring in parallel
    with weights on the sync ring; y is written f16 with one merged DMA
    per q-chunk (per-q-tile for the latency-critical last chunk) and
    partials are summed on host in fp32.

PSUM budget (8 banks): scores double-buffer 2x[128,2,512]f32 (4), ctx+den
accumulator [128,2,512]f32 (2), proj/out-proj/dup rotation 2x[128,512]f32
(2).
"""

import os
import numpy as np

import concourse.bass as bass
import concourse.bacc as bacc
import concourse.tile as tile
from concourse import mybir
from concourse.bass_utils import run_bass_kernel_spmd

F32 = mybir.dt.float32
F16 = mybir.dt.float16
ALU = mybir.AluOpType
ACTF = mybir.ActivationFunctionType

B, S_FULL, D, H = 2, 2048, 1024, 16
DK = 64              # head dim
JPC = 256            # j-dims (head dims) per core = 4 heads
N_JC = 2             # head pairs per core
N_KT = 16            # key tiles of 128
N_QC = 4             # query chunks of 512

LAST_RESULTS = None  # BassKernelResults of the most recent run (for test.py)


def build_nc(S=S_FULL):
    nc = bacc.Bacc("TRN2", target_bir_lowering=False, debug=False)

    # ---- DRAM I/O (per-core, host-prepped) ----
    xq = nc.dram_tensor("xq", [D, S], F16, kind="ExternalInput")   # q[b].T
    xk = nc.dram_tensor("xk", [D, S], F16, kind="ExternalInput")
    xv = nc.dram_tensor("xv", [D, S], F16, kind="ExternalInput")
    wq = nc.dram_tensor("wq", [D, JPC], F16, kind="ExternalInput")  # W_q[J,:].T
    wk = nc.dram_tensor("wk", [D, JPC], F16, kind="ExternalInput")
    wv = nc.dram_tensor("wv", [D, JPC], F16, kind="ExternalInput")
    wo = nc.dram_tensor("wo", [JPC, D], F16, kind="ExternalInput")  # W_o[:,J].T
    bq = nc.dram_tensor("bq", [128, N_JC], F32, kind="ExternalInput")
    y = nc.dram_tensor("y", [S, D], F16, kind="ExternalOutput")     # partial

    with tile.TileContext(nc) as tc:
        with (
            tc.tile_pool(name="consts", bufs=1) as consts,
            tc.tile_pool(name="persist", bufs=1) as persist,
            tc.tile_pool(name="ctxp", bufs=2) as ctxp,
            tc.tile_pool(name="xstream", bufs=10) as xstream,
            tc.tile_pool(name="apool", bufs=8) as apool,
            tc.tile_pool(name="recp", bufs=2) as recp,
            tc.tile_pool(name="ys", bufs=2) as ysp,
            tc.tile_pool(name="ps_st", bufs=2, space="PSUM") as ps_st,
            tc.tile_pool(name="ps_cd", bufs=1, space="PSUM") as ps_cd,
            tc.tile_pool(name="ps_io", bufs=2, space="PSUM") as ps_io,
        ):
            # ---- constants / weights (DMA emission order = priority) ----
            wk_sb = consts.tile([128, 8, JPC], F16, tag="wk")
            wq_sb = consts.tile([128, 8, JPC], F16, tag="wq")
            wv_sb = consts.tile([128, 8, JPC], F16, tag="wv")
            wo_sb = consts.tile([128, N_JC, D], F16, tag="wo")
            bq_sb = consts.tile([128, N_JC], F32, tag="bq")
            ones_sb = consts.tile([128, 64], F16, tag="ones")
            nc.vector.memset(ones_sb[:], 1.0)

            qt_sb = persist.tile([128, N_JC, S], F16, tag="qtp")   # Q^T [j, q]
            kt_sb = persist.tile([128, N_JC, S], F16, tag="ktp")   # K^T [j, k]
            v_sb = persist.tile([128, N_KT, JPC], F16, tag="vp")   # V [k, j]

            xq_r = xq.ap().rearrange("(c p) s -> p c s", p=128)
            xk_r = xk.ap().rearrange("(c p) s -> p c s", p=128)
            xv_r = xv.ap().rearrange("(c p) s -> p c s", p=128)
            wk_r = wk.ap().rearrange("(c p) j -> p c j", p=128)
            wq_r = wq.ap().rearrange("(c p) j -> p c j", p=128)
            wv_r = wv.ap().rearrange("(c p) j -> p c j", p=128)

            # DMA issue order (HWDGE FIFO = need order).  bq goes on the
            # SWDGE (gpsimd) queue: its [128,2] layout is 128 tiny
            # descriptors that would stall the main queue for ~2.7us.
            nc.sync.dma_start(out=wk_sb[:], in_=wk_r)
            x_t = {}
            x_t[("k", 0)] = xstream.tile([128, 8, 512], F16, tag="x",
                                         name="xk0")
            nc.scalar.dma_start(out=x_t[("k", 0)][:], in_=xk_r[:, :, 0:512])
            nc.sync.dma_start(out=wq_sb[:], in_=wq_r)
            x_t[("q", 0)] = xstream.tile([128, 8, 512], F16, tag="x",
                                         name="xq0")
            nc.scalar.dma_start(out=x_t[("q", 0)][:], in_=xq_r[:, :, 0:512])
            nc.gpsimd.dma_start(out=bq_sb[:], in_=bq.ap())
            for sc in (1, 2, 3):
                x_t[("k", sc)] = xstream.tile([128, 8, 512], F16, tag="x",
                                              name=f"xk{sc}")
                nc.sync.dma_start(out=x_t[("k", sc)][:],
                                  in_=xk_r[:, :, sc * 512:(sc + 1) * 512])
            nc.sync.dma_start(out=wv_sb[:], in_=wv_r)
            for n, sc in (("v", 0), ("q", 1), ("v", 1), ("q", 2), ("v", 2),
                          ("q", 3), ("v", 3)):
                src_r = xv_r if n == "v" else xq_r
                x_t[(n, sc)] = xstream.tile([128, 8, 512], F16, tag="x",
                                            name=f"x{n}{sc}")
                nc.sync.dma_start(out=x_t[(n, sc)][:],
                                  in_=src_r[:, :, sc * 512:(sc + 1) * 512])
            nc.sync.dma_start(
                out=wo_sb[:], in_=wo.ap().rearrange("(jc p) e -> p jc e", p=128))

            # ---- emission helpers (python emission order = engine order) --
            def emit_kproj(sc, jc):
                ps = ps_io.tile([128, 512], F32, tag="io",
                                name=f"kproj{sc}{jc}")
                for c in range(8):
                    nc.tensor.matmul(
                        ps[:], wk_sb[:, c, jc * 128:(jc + 1) * 128],
                        x_t[("k", sc)][:, c, :],
                        start=(c == 0), stop=(c == 7),
                    )
                nc.vector.tensor_copy(
                    kt_sb[:, jc, sc * 512:(sc + 1) * 512], ps[:])

            def emit_qproj(sc, jc):
                ps = ps_io.tile([128, 512], F32, tag="io",
                                name=f"qproj{sc}{jc}")
                for c in range(8):
                    nc.tensor.matmul(
                        ps[:], wq_sb[:, c, jc * 128:(jc + 1) * 128],
                        x_t[("q", sc)][:, c, :],
                        start=(c == 0), stop=(c == 7),
                    )
                nc.vector.tensor_scalar_add(
                    qt_sb[:, jc, sc * 512:(sc + 1) * 512], ps[:],
                    bq_sb[:, jc:jc + 1],
                )

            def emit_vproj(kt):
                sc, quarter = kt // 4, kt % 4
                ps = ps_io.tile([128, 512], F32, tag="io", name=f"vproj{kt}")
                for c in range(8):
                    nc.tensor.matmul(
                        ps[:, 0:JPC],
                        x_t[("v", sc)][:, c, quarter * 128:(quarter + 1) * 128],
                        wv_sb[:, c, :],
                        start=(c == 0), stop=(c == 7),
                    )
                nc.vector.tensor_copy(v_sb[:, kt, :], ps[:, 0:JPC])

            def emit_scores(qc, jc, kt, a_all):
                q0 = qc * 512
                st = ps_st.tile([128, 2, 512], F32, tag="st",
                                name=f"st{qc}{jc}{kt}")
                for h2 in range(2):
                    p0, p1 = h2 * 64, (h2 + 1) * 64
                    nc.tensor.matmul(
                        st[:, h2, :],
                        kt_sb[p0:p1, jc, kt * 128:(kt + 1) * 128],
                        qt_sb[p0:p1, jc, q0:q0 + 512],
                        start=True, stop=True,
                        skip_group_check=True,
                    )
                a = apool.tile([128, 2, 512], F16, tag="at",
                               name=f"a{qc}{jc}{kt}")
                nc.scalar.activation(a[:], st[:], ACTF.Exp, bias=0.0,
                                     scale=0.125)
                a_all[(qc * N_JC + jc) * N_KT + kt] = a

            def emit_av(qc, jc, kt, cd, a_all, blk_i):
                # ctx pair then den pair; each pair runs col-tiled
                # concurrently (disjoint column groups), den's 64-dup rows
                # land partition-aligned with ctx for the drain scale.
                a = a_all.pop(blk_i * N_KT + kt)
                for h2 in range(2):
                    nc.tensor.matmul(
                        cd[h2 * 64:(h2 + 1) * 64, 0, :],
                        v_sb[:, kt, jc * 128 + h2 * 64:jc * 128 + (h2 + 1) * 64],
                        a[:, h2, :],
                        start=(kt == 0), stop=(kt == N_KT - 1),
                        tile_position=(0, h2 * 64),
                        skip_group_check=True,
                    )
                for h2 in range(2):
                    nc.tensor.matmul(
                        cd[h2 * 64:(h2 + 1) * 64, 1, :],
                        ones_sb[:],
                        a[:, h2, :],
                        start=(kt == 0), stop=(kt == N_KT - 1),
                        tile_position=(0, h2 * 64),
                        skip_group_check=True,
                    )

            def emit_drain_main(qc, jc, cd, blk_i):
                rec = recp.tile([128, 512], F32, tag="rec",
                                name=f"rec{blk_i}")
                nc.vector.reciprocal_approx_fast(rec[:], cd[:, 1, :])
                nc.vector.tensor_tensor(
                    out=ctx_of[qc][:, jc, :], in0=cd[:, 0, :], in1=rec[:],
                    op=ALU.mult,
                )

            ys_of = {}

            def emit_outproj(qc, qt, ec):
                if qt == 0 and ec == 0:
                    ys_of[qc] = ysp.tile([128, 4, 1024], F16, tag="y",
                                         name=f"y{qc}")
                ysb = ys_of[qc]
                ps = ps_io.tile([128, 512], F32, tag="io",
                                name=f"yps{qc}{qt}{ec}")
                for jc in range(N_JC):
                    nc.tensor.matmul(
                        ps[:],
                        ctx_of[qc][:, jc, qt * 128:(qt + 1) * 128],
                        wo_sb[:, jc, ec * 512:(ec + 1) * 512],
                        start=(jc == 0), stop=(jc == N_JC - 1),
                    )
                if qc == N_QC - 1 and ec == 0:
                    nc.scalar.copy(ysb[:, qt, ec * 512:(ec + 1) * 512],
                                   ps[:])
                else:
                    nc.vector.tensor_copy(
                        ysb[:, qt, ec * 512:(ec + 1) * 512], ps[:])
                if qc == N_QC - 1:
                    if ec == 1:
                        qa = qc * 512 + qt * 128
                        nc.sync.dma_start(out=y.ap()[qa:qa + 128, :],
                                          in_=ysb[:, qt, :])
                elif qt == 3 and ec == 1:
                    nc.sync.dma_start(
                        out=y.ap()[qc * 512:(qc + 1) * 512, :].rearrange(
                            "(t p) e -> p t e", p=128),
                        in_=ysb[:])

            # =========== PE program: global-slot software pipeline ========
            blocks = [(qc, jc) for qc in range(N_QC) for jc in range(N_JC)]
            ctx_of = {}
            a_all = {}
            AV_LAG = 4

            inserts = {}

            def _add(s, fn, *args):
                inserts.setdefault(s, []).append((fn, args))

            _add(0, emit_kproj, 0, 1)
            _add(1, emit_kproj, 1, 0)
            _add(2, emit_kproj, 1, 1)
            _add(3, emit_qproj, 0, 1)
            _add(4, emit_kproj, 2, 0)
            _add(5, emit_kproj, 2, 1)
            _add(7, emit_kproj, 3, 0)
            _add(8, emit_kproj, 3, 1)
            for kt in range(N_KT):
                _add(kt + 3, emit_vproj, kt)
            _add(19, emit_qproj, 1, 0)
            _add(20, emit_qproj, 1, 1)
            _add(24, emit_qproj, 2, 0)
            _add(25, emit_qproj, 2, 1)
            _add(28, emit_qproj, 3, 0)
            _add(29, emit_qproj, 3, 1)
            # out-proj of qc: ctx128 complete shortly after the drain chain
            # of block 2qc+1; spread the 8 (qt, ec) pieces every 4 slots.
            for qc in range(N_QC - 1):
                for k in range(8):
                    _add((2 * qc + 2) * 16 + 5 + 4 * k, emit_outproj,
                         qc, k // 2, k % 2)

            av_sched = {}
            n_blocks = len(blocks)
            for i in range(n_blocks):
                base = N_KT * i
                if i < n_blocks - 1:
                    pairs = ([(6, (0, 1)), (7, (2, 3))] +
                             [(kt + 4, (kt,)) for kt in range(4, 16)])
                else:
                    pairs = ([(6, (0, 1)), (7, (2, 3))] +
                             [(kt + 4, (kt,)) for kt in range(4, 12)] +
                             [(16, (12, 13)), (17, (14, 15))])
                for off, akts in pairs:
                    av_sched.setdefault(base + off, []).extend(
                        (i, a) for a in akts)
            chain_sched = {}
            for i in range(n_blocks):
                chain_sched[N_KT * i + (20 if i < n_blocks - 1 else 18)] = i

            emit_kproj(0, 0)
            emit_qproj(0, 0)
            cd_of = {}
            n_tiles = n_blocks * N_KT
            for s in range(n_tiles + 7):
                if s < n_tiles:
                    i, kt = divmod(s, N_KT)
                    qc, jc = blocks[i]
                    if jc == 0 and kt == 0:
                        ctx_of[qc] = ctxp.tile([128, N_JC, 512], F16,
                                               tag="ctx", name=f"ctx{qc}")
                    emit_scores(qc, jc, kt, a_all)
                for fn, args in inserts.get(s, ()):
                    fn(*args)
                if s in chain_sched:
                    ai = chain_sched[s]
                    aqc, ajc = blocks[ai]
                    emit_drain_main(aqc, ajc, cd_of.pop(ai), ai)
                for ai, akt in av_sched.get(s, ()):
                    aqc, ajc = blocks[ai]
                    if akt == 0:
                        cd_of[ai] = ps_cd.tile([128, 2, 512], F32, tag="cd",
                                               name=f"cd{ai}")
                    emit_av(aqc, ajc, akt, cd_of[ai], a_all, ai)
            # tail: out-proj of qc3
            for qt in range(4):
                for ec in range(2):
                    emit_outproj(N_QC - 1, qt, ec)

    nc.compile()
    return nc


def shard_inputs(q, k, v, W_q, b_q, W_k, W_v, W_o):
    """Build per-core input maps. Core c: batch c//4, heads (c%4)*4..+4."""
    h = np.float16
    xq_b = [np.ascontiguousarray(q[b].T, dtype=h) for b in range(B)]
    xk_b = [np.ascontiguousarray(k[b].T, dtype=h) for b in range(B)]
    xv_b = [np.ascontiguousarray(v[b].T, dtype=h) for b in range(B)]
    in_maps = []
    for c in range(8):
        b = c // 4
        hp = c % 4
        J = slice(hp * JPC, (hp + 1) * JPC)
        m = {
            "xq": xq_b[b],
            "xk": xk_b[b],
            "xv": xv_b[b],
            "wq": np.ascontiguousarray(W_q[J, :].T, dtype=h),
            "wk": np.ascontiguousarray(W_k[J, :].T, dtype=h),
            "wv": np.ascontiguousarray(W_v[J, :].T, dtype=h),
            "wo": np.ascontiguousarray(W_o[:, J].T, dtype=h),
            "bq": np.ascontiguousarray(
                np.asarray(b_q[J], dtype=np.float32).reshape(N_JC, 128).T),
        }
        in_maps.append(m)
    return in_maps


def _enable_tracing():
    """Best-effort NTFF profiling under axon in this trimmed container:
    provide the antenv.axon_hooks module trn_boot expects, backed by the
    libaxon_pjrt.so profile C API, and stub out the S3 artifact upload.
    Only used when ATTN_TRACE=1 (never in the grading path)."""
    import sys
    import types
    import ctypes
    import contextlib

    try:
        import antenv.axon_hooks  # noqa: F401
        return
    except ImportError:
        pass

    holder = {"hook": None}
    mod = types.ModuleType("antenv.axon_hooks")
    mod.set_axon_ntff_profile_hook = lambda h: holder.__setitem__("hook", h)
    mod.get_axon_ntff_profile_hook = lambda: holder["hook"]
    sys.modules["antenv.axon_hooks"] = mod
    import antenv
    antenv.axon_hooks = mod

    so_path = "/opt/axon/libaxon_pjrt.so"
    if os.path.exists(so_path):
        lib = ctypes.CDLL(so_path)
        if hasattr(lib, "axon_start_nrt_profile"):
            lib.axon_start_nrt_profile.argtypes = [
                ctypes.POINTER(ctypes.c_int64), ctypes.c_size_t]
            lib.axon_start_nrt_profile.restype = ctypes.c_int64
            lib.axon_stop_nrt_profile.argtypes = [ctypes.c_char_p]
            lib.axon_stop_nrt_profile.restype = ctypes.c_int64

            @contextlib.contextmanager
            def _hook(output_dir, device_ids):
                import jax
                jax.devices()
                if device_ids:
                    ids = (ctypes.c_int64 * len(device_ids))(*device_ids)
                    rc = lib.axon_start_nrt_profile(ids, len(device_ids))
                else:
                    rc = lib.axon_start_nrt_profile(None, 0)
                if rc != 0:
                    raise RuntimeError(f"axon_start_nrt_profile rc={rc}")
                try:
                    yield
                finally:
                    n = lib.axon_stop_nrt_profile(str(output_dir).encode())
                    print(f"ntff profile: {n} file(s) -> {output_dir}")

            mod.set_axon_ntff_profile_hook(_hook)

    # upload_artifacts needs S3 creds we don't have; keep it local.
    import concourse.bass_utils as bu
    bu.upload_artifacts = lambda tmpdir: tmpdir


_NC_CACHE = {}


def kernel(q, k, v, mask, W_q, b_q, W_k, b_k, W_v, b_v, W_o, b_o):
    """Full-input, full-output attention. mask is all-ones (unused)."""
    global LAST_RESULTS
    q = np.asarray(q, np.float32)
    k = np.asarray(k, np.float32)
    v = np.asarray(v, np.float32)
    W_q = np.asarray(W_q, np.float32)
    W_k = np.asarray(W_k, np.float32)
    W_v = np.asarray(W_v, np.float32)
    W_o = np.asarray(W_o, np.float32)
    b_v = np.asarray(b_v, np.float32)
    b_o = np.asarray(b_o, np.float32)

    if "nc" not in _NC_CACHE:
        _NC_CACHE["nc"] = build_nc(S_FULL)
    nc = _NC_CACHE["nc"]

    in_maps = shard_inputs(q, k, v, W_q, b_q, W_k, W_v, W_o)
    trace = bool(int(os.environ.get("ATTN_TRACE", "0")))
    if trace:
        _enable_tracing()
    res = run_bass_kernel_spmd(nc, in_maps, list(range(8)), trace=trace)
    LAST_RESULTS = res

    out = np.zeros((B, S_FULL, D), np.float32)
    for c in range(8):
        out[c // 4] += np.asarray(res.results[c]["y"], np.float32)
    # b_o plus the folded-out value bias: ctx = ctx' + b_v  =>  + b_v @ W_o.T
    out += np.asarray(b_o, np.float32) + W_o @ b_v
    return out
